# revision 41
# baseline (speedup 1.0000x reference)
"""AttentionContext kernel for Trainium2, data-parallel over batch on 8 cores.

Reference computation (B=64, T=2048, D=512 everywhere):
    phi_s = s @ phi_w.T + phi_b                  # [B, D]
    psi_h = einsum('bth,ah->bta', h, psi_w) + psi_b
    e     = einsum('ba,bta->bt', phi_s, psi_h)   # [B, T]
    alpha = softmax(e, axis=-1)
    c     = alpha * h.sum(-1)                    # [B, T]

Algebraic restructuring:
    e[b,t] = (phi_s[b] @ psi_w) . h[b,t] + const(b); softmax drops const(b).
    w = s @ (phi_w.T @ psi_w) + phi_b @ psi_w    # [B, D], tiny, on PE

Streaming design (per core: 8 batches, 128 tiles of [128, 512]):
  * All DRAM traffic rides one SWDGE (Pool) queue: small fp16-cast weight
    loads first (HWDGE weight loads starve behind a saturated SWDGE h
    stream at the shared SDMA engines), then the h stream. h is cast
    fp32->fp16 inline in the DMA; the t-mapping t = p*16 + j makes each
    partition's slice of a batch one contiguous 32KB DRAM read. Softmax is
    permutation-invariant in t and the output store needs no transpose.
  * The free-dim reductions (e = sum_d h*w and hsum = sum_d h) ride the
    TensorEngine: with a stationary fp16 identity, 16 accumulating
    matmuls per batch fold [128, 16, 512] -> PSUM [128, 16, 32] (each
    rhs chunk is an identity-copy accumulated into the same PSUM bank,
    fp32). One segmented DVE tensor_reduce finishes 32 -> 1 straight out
    of PSUM. The DVE only computes the h*w product (fp16 tensor_tensor at
    2 elem/cycle against a stride-0-broadcast w), in halves interleaved
    with the PE folds to keep the PE clock-gate (HAM) warm.
  * stage 0 runs in fp16 on a HAM-warmed PE so w is broadcast (K=1 matmul
    per batch + PSUM evacuation) before the first batch finishes loading.
    The last batch loads in quarter tiles and folds per quarter to keep
    the post-stream tail short.
  * softmax: per-batch exp(e - colmax_p) on ScalarE with the row max as
    activation bias (negate=True reduce), then one batched cross-partition
    combine: bmax via PE transpose + reduce, correction exp(colmax - bmax)
    folded into the final normalization multiply, single 64KB store.
"""

import numpy as np

import concourse.bass as bass
import concourse.bacc as bacc
import concourse.tile as tile
from concourse import mybir
from concourse import bass_utils
from concourse.masks import make_identity

FP = mybir.dt.float32
F16 = mybir.dt.float16
ALU = mybir.AluOpType
AF = mybir.ActivationFunctionType

N_CORES = 8
B_LOC = 8          # batches per core
T = 2048
D = 512
P = 128
KC = D // P        # 4 contraction chunks of 128
NJ = T // P        # 16 t-tiles per batch (t = p*16 + j)
QW = 32            # PE fold output width: 512 -> QW via D//QW matmuls


def _rep_ap(ap2, n):
    """[P, W] AP -> [P, n, W] view with stride-0 middle dim."""
    return bass.AP(
        tensor=ap2.tensor, offset=ap2.offset, ap=[ap2.ap[0], [0, n], ap2.ap[1]]
    )


def _emit(nc, tc):
    s = nc.dram_tensor("s", [B_LOC, D], FP, kind="ExternalInput").ap()
    h = nc.dram_tensor("h", [B_LOC, T, D], FP, kind="ExternalInput").ap()
    phi_w = nc.dram_tensor("phi_w", [D, D], FP, kind="ExternalInput").ap()
    phi_b = nc.dram_tensor("phi_b", [D], FP, kind="ExternalInput").ap()
    psi_w = nc.dram_tensor("psi_w", [D, D], FP, kind="ExternalInput").ap()
    c_out = nc.dram_tensor("c", [B_LOC, T], FP, kind="ExternalOutput").ap()

    with tc.tile_pool(name="consts", bufs=1) as consts:
        # ------- stage 0 inputs: fp16 cast loads, FIRST on the Pool queue ----
        s_sb = consts.tile([B_LOC, D], FP)
        nc.gpsimd.dma_start(out=s_sb, in_=s)
        phi16_sb = consts.tile([P, KC, D], F16)  # [a % 128, a // 128, k]
        nc.gpsimd.dma_start(
            out=phi16_sb, in_=phi_w.rearrange("(ac p) k -> p ac k", p=P)
        )
        psi16_sb = consts.tile([P, KC, D], F16)  # [a % 128, a // 128, m]
        nc.gpsimd.dma_start(
            out=psi16_sb, in_=psi_w.rearrange("(ac p) k -> p ac k", p=P)
        )
        phi_b16 = consts.tile([P, KC], F16)      # [a % 128, a // 128]
        nc.gpsimd.dma_start(
            out=phi_b16, in_=phi_b.rearrange("(ac p) -> p ac", p=P)
        )

        # identity after the weight dispatches but before the h loads:
        # its GpSimd ops ride the Pool queue while the weight transfers
        # stream, and ident is ready (~9us) well before the PE warm-up
        ident = consts.tile([P, P], FP)
        make_identity(nc, ident)
        ident16 = consts.tile([P, P], F16)
        nc.vector.tensor_copy(out=ident16, in_=ident)
        ones_1x128 = consts.tile([1, P], FP)
        nc.vector.memset(ones_1x128, 1.0)
        neg_1x128 = consts.tile([1, P], FP)
        nc.vector.memset(neg_1x128, -1.0)
        ones_128x1 = consts.tile([P, 1], FP)
        nc.vector.memset(ones_128x1, 1.0)
        ones16_1x128 = consts.tile([1, P], F16)
        nc.vector.memset(ones16_1x128, 1.0)

        # Warm the ACT exp table set early so the ~2.7us load overlaps.
        tiny = consts.tile([1, 1], FP)
        nc.vector.memset(tiny, 0.0)
        nc.scalar.activation(out=tiny, in_=tiny, func=AF.Exp)

        mc_sb = consts.tile([P, KC, D], F16)     # M_c[k, m], k = kc*128 + p
        v_sb = consts.tile([1, D], F16)          # v[m] = phi_b @ psi_w
        sT_sb = consts.tile([P, KC, B_LOC], F16)  # s.T[k, b]
        w_sb16 = consts.tile([B_LOC, D], F16)    # w[b, m] fp16
        w_rows16 = consts.tile([1, B_LOC, D], F16)  # each w row at partition 0
        w_bc16 = consts.tile([P, B_LOC, D], F16)  # w[b] broadcast down parts

        e_all = consts.tile([P, P], FP)          # e[p, b*16 + j], t = p*16+j
        hs_all = consts.tile([P, P], FP)         # hsum, same layout
        exp_all = consts.tile([P, P], FP)        # exp(e - colmax_p)
        ncm_all = consts.tile([P, B_LOC], FP)    # -colmax[p, b]
        pscol_all = consts.tile([P, B_LOC], FP)  # sum_j exp_all per (p, b)
        cmarg = consts.tile([P, B_LOC], FP)
        cmexp = consts.tile([P, B_LOC], FP)
        pscw = consts.tile([P, B_LOC], FP)
        nb_sb = consts.tile([P, B_LOC], FP)
        bmax_sb = consts.tile([B_LOC, 1], FP)
        bt_sb = consts.tile([1, B_LOC], FP)
        rcp_sb = consts.tile([B_LOC, 1], FP)
        rt_sb = consts.tile([1, B_LOC], FP)
        rb_sb = consts.tile([P, B_LOC], FP)
        wf_sb = consts.tile([P, B_LOC], FP)
        cbuf = consts.tile([P, P], FP)

        # HAM warm-up: back-to-back REAL matmuls (transpose-mode does not
        # count as PE-busy for HAM) flip the clock gate to 8/8 and keep it
        # there until the weight loads land (~17us), so the stage-0 chain
        # runs at 2.4GHz; results are discarded.
        with tc.tile_pool(name="psumw", bufs=2, space="PSUM") as psumw:
            for _ in range(26):
                warm_ps = psumw.tile([P, P], FP, tag="warm_ps")
                nc.tensor.matmul(warm_ps, lhsT=ident16, rhs=ident16)

        with tc.tile_pool(name="psum0", bufs=2, space="PSUM") as psum0:
            for kc in range(KC):
                st_ps = psum0.tile([P, B_LOC], FP, tag="st_ps")
                nc.tensor.transpose(
                    st_ps,
                    in_=s_sb[:, kc * P : (kc + 1) * P],
                    identity=ident[:B_LOC, :B_LOC],
                )
                nc.vector.tensor_copy(out=sT_sb[:, kc, :], in_=st_ps)

            # M_c[k, m] = sum_a phi_w[a, k] * psi_w[a, m]   (fp16 operands)
            for kc in range(KC):
                mc_ps = psum0.tile([P, D], FP, tag="mc_ps")
                for ac in range(KC):
                    nc.tensor.matmul(
                        mc_ps,
                        lhsT=phi16_sb[:, ac, kc * P : (kc + 1) * P],
                        rhs=psi16_sb[:, ac, :],
                        start=(ac == 0),
                        stop=(ac == KC - 1),
                    )
                nc.vector.tensor_copy(out=mc_sb[:, kc, :], in_=mc_ps)

            # v[m] = sum_a phi_b[a] * psi_w[a, m]
            v_ps = psum0.tile([1, D], FP, tag="v_ps")
            for ac in range(KC):
                nc.tensor.matmul(
                    v_ps,
                    lhsT=phi_b16[:, ac : ac + 1],
                    rhs=psi16_sb[:, ac, :],
                    start=(ac == 0),
                    stop=(ac == KC - 1),
                )
            nc.vector.tensor_copy(out=v_sb, in_=v_ps)

            # w[b, m] = sum_k sT[k, b] * M_c[k, m] + 1 * v[m]
            w_ps = psum0.tile([B_LOC, D], FP, tag="w_ps")
            for kc in range(KC):
                nc.tensor.matmul(
                    w_ps,
                    lhsT=sT_sb[:, kc, :],
                    rhs=mc_sb[:, kc, :],
                    start=(kc == 0),
                    stop=False,
                )
            nc.tensor.matmul(
                w_ps, lhsT=ones16_1x128[:, :B_LOC], rhs=v_sb,
                start=False, stop=True,
            )
            nc.vector.tensor_copy(out=w_sb16, in_=w_ps)  # cast fp32 -> fp16

        # relocate all w rows to partition 0 in ONE SBUF->SBUF DMA (each
        # tiny DMA pays ~2.2us of completion latency, so one beats eight;
        # PE matmul rhs must start at partition 0/32/64). The broadcast
        # matmuls + evacuations are emitted inside the batch loop so the
        # first e-product doesn't queue behind all eight evacuations.
        nc.sync.dma_start(out=w_rows16, in_=w_sb16)

        # ---------------- stage 1: stream h ----------------
        with (
            tc.tile_pool(name="hpool", bufs=4) as hpool,
            tc.tile_pool(name="qpool", bufs=4) as qpool,
            tc.tile_pool(name="ppool", bufs=2) as ppool,
            tc.tile_pool(name="psum3", bufs=3, space="PSUM") as psum3,
            tc.tile_pool(name="psum1", bufs=2, space="PSUM") as psum1,
        ):
            QN = NJ // 4

            def _fold(src3, n, tag):
                """[P, n, 512] -> PSUM [P, n, 64] via 8 accumulating
                identity matmuls."""
                ps = psum3.tile([P, NJ // 2, 64], FP, tag=tag)
                for ci in range(8):
                    nc.tensor.matmul(
                        ps[:, :n, :], lhsT=ident16,
                        rhs=src3[:, :, ci * 64 : (ci + 1) * 64],
                        start=(ci == 0), stop=(ci == 7),
                    )
                return ps

            def do_batch(srcs, b, j0, n):
                """hsum + e for blocks of n tiles starting at j0. Emits all
                products (DVE) then all PE folds; RETURNS a closure that
                emits the DVE reduces (PSUM -> e_all/hs_all directly). The
                caller runs it after the NEXT batch's products, so a
                product never queues behind a reduce that waits on PE
                folds — the FIFO chain that paced v10 at ~13us/batch."""
                prods = []
                for src3 in srcs:
                    prod = ppool.tile([P, n, D], F16, tag=f"prod{n}")
                    nc.vector.tensor_tensor(
                        out=prod, in0=src3,
                        in1=_rep_ap(w_bc16[:, b, :], n), op=ALU.mult,
                    )
                    prods.append(prod)
                hs_list = [_fold(s, n, "hs_ps") for s in srcs]
                e_list = [_fold(p, n, "e_ps") for p in prods]

                def reduces():
                    for v, ps in enumerate(hs_list):
                        cols = slice(
                            b * NJ + j0 + v * n, b * NJ + j0 + (v + 1) * n
                        )
                        nc.vector.tensor_reduce(
                            out=hs_all[:, cols], in_=ps[:, :n, :],
                            axis=mybir.AxisListType.X, op=ALU.add,
                        )
                    for v, ps in enumerate(e_list):
                        cols = slice(
                            b * NJ + j0 + v * n, b * NJ + j0 + (v + 1) * n
                        )
                        nc.vector.tensor_reduce(
                            out=e_all[:, cols], in_=ps[:, :n, :],
                            axis=mybir.AxisListType.X, op=ALU.add,
                        )

                return reduces

            # broadcast each w[b] down the partitions (K=1 fp16 matmuls, all
            # early on the PE) and evacuate on ScalarE, whose queue holds
            # only the much-later exps — so neither the PE fold stream nor
            # the DVE product stream ever waits behind an evacuation.
            for b in range(B_LOC):
                bc_ps = psum1.tile([P, D], FP, tag="bc_ps")
                nc.tensor.matmul(bc_ps, lhsT=ones16_1x128, rhs=w_rows16[:, b, :])
                nc.scalar.copy(out=w_bc16[:, b, :], in_=bc_ps)

            def finish_batch(b):
                """row max + exp(e - colmax_p); needs batch b's e columns"""
                c0 = b * NJ
                nc.vector.tensor_reduce(
                    out=ncm_all[:, b : b + 1], in_=e_all[:, c0 : c0 + NJ],
                    axis=mybir.AxisListType.X, op=ALU.max, negate=True,
                )
                nc.scalar.activation(
                    out=exp_all[:, c0 : c0 + NJ], in_=e_all[:, c0 : c0 + NJ],
                    func=AF.Exp, bias=ncm_all[:, b : b + 1], scale=1.0,
                    accum_out=pscol_all[:, b : b + 1],
                )

            pending = []  # [(reduce_closure, batch_or_None_to_finish)]
            for b in range(B_LOC):
                last = b == B_LOC - 1
                if last:
                    # shrinking blocks [8, 4, 2, 2] in separate tiles:
                    # precise deps + a short critical chain after the very
                    # last (small) block lands
                    h3 = h[b].rearrange("(p j) d -> p j d", p=P)
                    sizes = [8, 4, 2, 2]
                    hqs = []
                    j0 = 0
                    for v, nq in enumerate(sizes):
                        hq = qpool.tile([P, nq, D], F16, tag=f"hq{v}")
                        nc.gpsimd.dma_start(
                            out=hq, in_=h3[:, j0 : j0 + nq, :]
                        )
                        hqs.append((hq, j0, nq))
                        j0 += nq
                    r1 = do_batch([hqs[0][0]], b, hqs[0][1], hqs[0][2])
                    for red, fb in pending:
                        red()
                        if fb is not None:
                            finish_batch(fb)
                    pending = []
                    r2 = do_batch([hqs[1][0]], b, hqs[1][1], hqs[1][2])
                    r1()
                    r3 = do_batch([hqs[2][0]], b, hqs[2][1], hqs[2][2])
                    r2()
                    r4 = do_batch([hqs[3][0]], b, hqs[3][1], hqs[3][2])
                    r3()
                    r4()
                    finish_batch(b)
                else:
                    ht = hpool.tile([P, NJ, D], F16, tag="ht")
                    nc.gpsimd.dma_start(
                        out=ht, in_=h[b].rearrange("(p j) d -> p j d", p=P)
                    )
                    red = do_batch(
                        [ht[:, 0 : NJ // 2, :], ht[:, NJ // 2 :, :]],
                        b, 0, NJ // 2,
                    )
                    for r, fb in pending:
                        r()
                        if fb is not None:
                            finish_batch(fb)
                    pending = [(red, b)]

        # ---------------- stage 2: batched softmax combine ----------------
        with tc.tile_pool(name="psum2", bufs=1, space="PSUM") as psum2:
            # bmax[b] = max_p colmax[p, b]; ncm = -colmax
            cmT_ps = psum2.tile([B_LOC, P], FP, tag="cmT_ps")
            nc.tensor.transpose(cmT_ps, in_=ncm_all, identity=ident)
            nc.vector.tensor_reduce(
                out=bmax_sb, in_=cmT_ps, axis=mybir.AxisListType.X,
                op=ALU.min, negate=True,
            )
            bt_ps = psum2.tile([1, B_LOC], FP, tag="bt_ps")
            nc.tensor.transpose(
                bt_ps, in_=bmax_sb, identity=ident[:B_LOC, :B_LOC]
            )
            nc.vector.tensor_copy(out=bt_sb, in_=bt_ps)
            # -bmax broadcast down partitions
            nb_ps = psum2.tile([P, B_LOC], FP, tag="nb_ps")
            nc.tensor.matmul(nb_ps, lhsT=neg_1x128, rhs=bt_sb)
            nc.vector.tensor_copy(out=nb_sb, in_=nb_ps)
            # cmarg = colmax - bmax = nb - ncm
            nc.vector.tensor_tensor(
                out=cmarg, in0=nb_sb, in1=ncm_all, op=ALU.subtract
            )
            nc.scalar.activation(out=cmexp, in_=cmarg, func=AF.Exp)
            nc.vector.tensor_tensor(
                out=pscw, in0=pscol_all, in1=cmexp, op=ALU.mult
            )
            # Z[b] = sum_p pscw[p, b]
            z_ps = psum2.tile([B_LOC, 1], FP, tag="z_ps")
            nc.tensor.matmul(z_ps, lhsT=pscw, rhs=ones_128x1)
            nc.vector.reciprocal(out=rcp_sb, in_=z_ps)
            rt_ps = psum2.tile([1, B_LOC], FP, tag="rt_ps")
            nc.tensor.transpose(
                rt_ps, in_=rcp_sb, identity=ident[:B_LOC, :B_LOC]
            )
            nc.vector.tensor_copy(out=rt_sb, in_=rt_ps)
            rb_ps = psum2.tile([P, B_LOC], FP, tag="rb_ps")
            nc.tensor.matmul(rb_ps, lhsT=ones_1x128, rhs=rt_sb)
            nc.vector.tensor_copy(out=rb_sb, in_=rb_ps)
            nc.vector.tensor_tensor(out=wf_sb, in0=cmexp, in1=rb_sb, op=ALU.mult)

            # c = exp_all * hs_all * wf[p, b]  (wf broadcast over j)
            nc.vector.tensor_tensor(
                out=cbuf, in0=exp_all, in1=hs_all, op=ALU.mult
            )
            wf_rep = bass.AP(
                tensor=wf_sb.tensor, offset=wf_sb.offset,
                ap=[wf_sb.ap[0], wf_sb.ap[1], [0, NJ]],
            )
            cbuf3 = bass.AP(
                tensor=cbuf.tensor, offset=cbuf.offset,
                ap=[cbuf.ap[0], [NJ, B_LOC], [1, NJ]],
            )
            nc.vector.tensor_tensor(
                out=cbuf3, in0=cbuf3, in1=wf_rep, op=ALU.mult
            )
            # c[b, p*16 + j] = cbuf[p, b*16 + j]. The DRAM pattern is 1024
            # 64B segments (RMW-penalized), so split across both HWDGE
            # rings plus the (now idle) SWDGE ring to cut the serial time.
            for lane, (engine, b0r, b1r) in enumerate(
                [(nc.sync, 0, 3), (nc.scalar, 3, 6), (nc.gpsimd, 6, 8)]
            ):
                engine.dma_start(
                    out=c_out[b0r:b1r].rearrange("b (p j) -> p b j", p=P),
                    in_=cbuf[:, b0r * NJ : b1r * NJ],
                )


_CACHE = {}


def _build():
    if "nc" not in _CACHE:
        nc = bacc.Bacc(
            "TRN2", target_bir_lowering=False, debug=False, num_devices=N_CORES
        )
        with tile.TileContext(nc) as tc:
            _emit(nc, tc)
        nc.compile()
        _CACHE["nc"] = nc
    return _CACHE["nc"]


def kernel(s, h, phi_w, phi_b, psi_w, psi_b=None, **_unused):
    s = np.ascontiguousarray(np.asarray(s, dtype=np.float32))
    h = np.ascontiguousarray(np.asarray(h, dtype=np.float32))
    phi_w = np.ascontiguousarray(np.asarray(phi_w, dtype=np.float32))
    phi_b = np.ascontiguousarray(np.asarray(phi_b, dtype=np.float32))
    psi_w = np.ascontiguousarray(np.asarray(psi_w, dtype=np.float32))

    nc = _build()
    in_maps = [
        {
            "s": s[i * B_LOC : (i + 1) * B_LOC],
            "h": h[i * B_LOC : (i + 1) * B_LOC],
            "phi_w": phi_w,
            "phi_b": phi_b,
            "psi_w": psi_w,
        }
        for i in range(N_CORES)
    ]
    res = bass_utils.run_bass_kernel_spmd(nc, in_maps, core_ids=list(range(N_CORES)))
    return np.concatenate(
        [res.results[i]["c"] for i in range(N_CORES)], axis=0
    ).astype(np.float32)


# revision 42
# speedup vs baseline: 1.0027x; 1.0027x over previous
"""AttentionContext kernel for Trainium2, data-parallel over batch on 8 cores.

Reference computation (B=64, T=2048, D=512 everywhere):
    phi_s = s @ phi_w.T + phi_b                  # [B, D]
    psi_h = einsum('bth,ah->bta', h, psi_w) + psi_b
    e     = einsum('ba,bta->bt', phi_s, psi_h)   # [B, T]
    alpha = softmax(e, axis=-1)
    c     = alpha * h.sum(-1)                    # [B, T]

Algebraic restructuring:
    e[b,t] = (phi_s[b] @ psi_w) . h[b,t] + const(b); softmax drops const(b).
    w = s @ (phi_w.T @ psi_w) + phi_b @ psi_w    # [B, D], tiny, on PE

Streaming design (per core: 8 batches, 128 tiles of [128, 512]):
  * All DRAM traffic rides one SWDGE (Pool) queue: small fp16-cast weight
    loads first (HWDGE weight loads starve behind a saturated SWDGE h
    stream at the shared SDMA engines), then the h stream. h is cast
    fp32->fp16 inline in the DMA; the t-mapping t = p*16 + j makes each
    partition's slice of a batch one contiguous 32KB DRAM read. Softmax is
    permutation-invariant in t and the output store needs no transpose.
  * The free-dim reductions (e = sum_d h*w and hsum = sum_d h) ride the
    TensorEngine: with a stationary fp16 identity, 16 accumulating
    matmuls per batch fold [128, 16, 512] -> PSUM [128, 16, 32] (each
    rhs chunk is an identity-copy accumulated into the same PSUM bank,
    fp32). One segmented DVE tensor_reduce finishes 32 -> 1 straight out
    of PSUM. The DVE only computes the h*w product (fp16 tensor_tensor at
    2 elem/cycle against a stride-0-broadcast w), in halves interleaved
    with the PE folds to keep the PE clock-gate (HAM) warm.
  * stage 0 runs in fp16 on a HAM-warmed PE so w is broadcast (K=1 matmul
    per batch + PSUM evacuation) before the first batch finishes loading.
    The last batch loads in quarter tiles and folds per quarter to keep
    the post-stream tail short.
  * softmax: per-batch exp(e - colmax_p) on ScalarE with the row max as
    activation bias (negate=True reduce), then one batched cross-partition
    combine: bmax via PE transpose + reduce, correction exp(colmax - bmax)
    folded into the final normalization multiply, single 64KB store.
"""

import numpy as np

import concourse.bass as bass
import concourse.bacc as bacc
import concourse.tile as tile
from concourse import mybir
from concourse import bass_utils
from concourse.masks import make_identity

FP = mybir.dt.float32
F16 = mybir.dt.float16
ALU = mybir.AluOpType
AF = mybir.ActivationFunctionType

N_CORES = 8
B_LOC = 8          # batches per core
T = 2048
D = 512
P = 128
KC = D // P        # 4 contraction chunks of 128
NJ = T // P        # 16 t-tiles per batch (t = p*16 + j)
QW = 32            # PE fold output width: 512 -> QW via D//QW matmuls


def _rep_ap(ap2, n):
    """[P, W] AP -> [P, n, W] view with stride-0 middle dim."""
    return bass.AP(
        tensor=ap2.tensor, offset=ap2.offset, ap=[ap2.ap[0], [0, n], ap2.ap[1]]
    )


def _emit(nc, tc):
    s = nc.dram_tensor("s", [B_LOC, D], FP, kind="ExternalInput").ap()
    h = nc.dram_tensor("h", [B_LOC, T, D], FP, kind="ExternalInput").ap()
    phi_w = nc.dram_tensor("phi_w", [D, D], FP, kind="ExternalInput").ap()
    phi_b = nc.dram_tensor("phi_b", [D], FP, kind="ExternalInput").ap()
    psi_w = nc.dram_tensor("psi_w", [D, D], FP, kind="ExternalInput").ap()
    c_out = nc.dram_tensor("c", [B_LOC, T], FP, kind="ExternalOutput").ap()

    with tc.tile_pool(name="consts", bufs=1) as consts:
        # ------- stage 0 inputs: fp16 cast loads, FIRST on the Pool queue ----
        s_sb = consts.tile([B_LOC, D], FP)
        nc.gpsimd.dma_start(out=s_sb, in_=s)
        phi16_sb = consts.tile([P, KC, D], F16)  # [a % 128, a // 128, k]
        nc.gpsimd.dma_start(
            out=phi16_sb, in_=phi_w.rearrange("(ac p) k -> p ac k", p=P)
        )
        psi16_sb = consts.tile([P, KC, D], F16)  # [a % 128, a // 128, m]
        nc.gpsimd.dma_start(
            out=psi16_sb, in_=psi_w.rearrange("(ac p) k -> p ac k", p=P)
        )
        phi_b16 = consts.tile([P, KC], F16)      # [a % 128, a // 128]
        nc.gpsimd.dma_start(
            out=phi_b16, in_=phi_b.rearrange("(ac p) -> p ac", p=P)
        )

        # identity after the weight dispatches but before the h loads:
        # its GpSimd ops ride the Pool queue while the weight transfers
        # stream, and ident is ready (~9us) well before the PE warm-up
        ident = consts.tile([P, P], FP)
        make_identity(nc, ident)
        ident16 = consts.tile([P, P], F16)
        nc.vector.tensor_copy(out=ident16, in_=ident)
        ones_1x128 = consts.tile([1, P], FP)
        nc.vector.memset(ones_1x128, 1.0)
        neg_1x128 = consts.tile([1, P], FP)
        nc.vector.memset(neg_1x128, -1.0)
        ones_128x1 = consts.tile([P, 1], FP)
        nc.vector.memset(ones_128x1, 1.0)
        ones16_1x128 = consts.tile([1, P], F16)
        nc.vector.memset(ones16_1x128, 1.0)

        # Warm the ACT exp table set early so the ~2.7us load overlaps.
        tiny = consts.tile([1, 1], FP)
        nc.vector.memset(tiny, 0.0)
        nc.scalar.activation(out=tiny, in_=tiny, func=AF.Exp)

        mc_sb = consts.tile([P, KC, D], F16)     # M_c[k, m], k = kc*128 + p
        v_sb = consts.tile([1, D], F16)          # v[m] = phi_b @ psi_w
        sT_sb = consts.tile([P, KC, B_LOC], F16)  # s.T[k, b]
        w_sb16 = consts.tile([B_LOC, D], F16)    # w[b, m] fp16
        w_rows16 = consts.tile([1, B_LOC, D], F16)  # each w row at partition 0
        w_bc16 = consts.tile([P, B_LOC, D], F16)  # w[b] broadcast down parts

        e_all = consts.tile([P, P], FP)          # e[p, b*16 + j], t = p*16+j
        hs_all = consts.tile([P, P], FP)         # hsum, same layout
        exp_all = consts.tile([P, P], FP)        # exp(e - colmax_p)
        ncm_all = consts.tile([P, B_LOC], FP)    # -colmax[p, b]
        pscol_all = consts.tile([P, B_LOC], FP)  # sum_j exp_all per (p, b)
        cmarg = consts.tile([P, B_LOC], FP)
        cmexp = consts.tile([P, B_LOC], FP)
        pscw = consts.tile([P, B_LOC], FP)
        nb_sb = consts.tile([P, B_LOC], FP)
        bmax_sb = consts.tile([B_LOC, 1], FP)
        bt_sb = consts.tile([1, B_LOC], FP)
        rcp_sb = consts.tile([B_LOC, 1], FP)
        rt_sb = consts.tile([1, B_LOC], FP)
        rb_sb = consts.tile([P, B_LOC], FP)
        wf_sb = consts.tile([P, B_LOC], FP)
        cbuf = consts.tile([P, P], FP)

        # HAM warm-up: back-to-back REAL matmuls (transpose-mode does not
        # count as PE-busy for HAM) flip the clock gate to 8/8 and keep it
        # there until the weight loads land (~17us), so the stage-0 chain
        # runs at 2.4GHz; results are discarded.
        with tc.tile_pool(name="psumw", bufs=2, space="PSUM") as psumw:
            for _ in range(26):
                warm_ps = psumw.tile([P, P], FP, tag="warm_ps")
                nc.tensor.matmul(warm_ps, lhsT=ident16, rhs=ident16)

        with tc.tile_pool(name="psum0", bufs=2, space="PSUM") as psum0:
            for kc in range(KC):
                st_ps = psum0.tile([P, B_LOC], FP, tag="st_ps")
                nc.tensor.transpose(
                    st_ps,
                    in_=s_sb[:, kc * P : (kc + 1) * P],
                    identity=ident[:B_LOC, :B_LOC],
                )
                nc.vector.tensor_copy(out=sT_sb[:, kc, :], in_=st_ps)

            # M_c[k, m] = sum_a phi_w[a, k] * psi_w[a, m]   (fp16 operands)
            for kc in range(KC):
                mc_ps = psum0.tile([P, D], FP, tag="mc_ps")
                for ac in range(KC):
                    nc.tensor.matmul(
                        mc_ps,
                        lhsT=phi16_sb[:, ac, kc * P : (kc + 1) * P],
                        rhs=psi16_sb[:, ac, :],
                        start=(ac == 0),
                        stop=(ac == KC - 1),
                    )
                nc.vector.tensor_copy(out=mc_sb[:, kc, :], in_=mc_ps)

            # v[m] = sum_a phi_b[a] * psi_w[a, m]
            v_ps = psum0.tile([1, D], FP, tag="v_ps")
            for ac in range(KC):
                nc.tensor.matmul(
                    v_ps,
                    lhsT=phi_b16[:, ac : ac + 1],
                    rhs=psi16_sb[:, ac, :],
                    start=(ac == 0),
                    stop=(ac == KC - 1),
                )
            nc.vector.tensor_copy(out=v_sb, in_=v_ps)

            # w[b, m] = sum_k sT[k, b] * M_c[k, m] + 1 * v[m]
            w_ps = psum0.tile([B_LOC, D], FP, tag="w_ps")
            for kc in range(KC):
                nc.tensor.matmul(
                    w_ps,
                    lhsT=sT_sb[:, kc, :],
                    rhs=mc_sb[:, kc, :],
                    start=(kc == 0),
                    stop=False,
                )
            nc.tensor.matmul(
                w_ps, lhsT=ones16_1x128[:, :B_LOC], rhs=v_sb,
                start=False, stop=True,
            )
            nc.vector.tensor_copy(out=w_sb16, in_=w_ps)  # cast fp32 -> fp16

        # relocate all w rows to partition 0 in ONE SBUF->SBUF DMA (each
        # tiny DMA pays ~2.2us of completion latency, so one beats eight;
        # PE matmul rhs must start at partition 0/32/64). The broadcast
        # matmuls + evacuations are emitted inside the batch loop so the
        # first e-product doesn't queue behind all eight evacuations.
        nc.sync.dma_start(out=w_rows16, in_=w_sb16)

        # ---------------- stage 1: stream h ----------------
        with (
            tc.tile_pool(name="hpool", bufs=4) as hpool,
            tc.tile_pool(name="qpool", bufs=4) as qpool,
            tc.tile_pool(name="ppool", bufs=2) as ppool,
            tc.tile_pool(name="psum3", bufs=3, space="PSUM") as psum3,
            tc.tile_pool(name="psum1", bufs=2, space="PSUM") as psum1,
        ):
            QN = NJ // 4

            def _fold(src3, n, tag):
                """[P, n, 512] -> PSUM [P, n, 64] via 8 accumulating
                identity matmuls."""
                ps = psum3.tile([P, NJ // 2, 64], FP, tag=tag)
                for ci in range(8):
                    nc.tensor.matmul(
                        ps[:, :n, :], lhsT=ident16,
                        rhs=src3[:, :, ci * 64 : (ci + 1) * 64],
                        start=(ci == 0), stop=(ci == 7),
                    )
                return ps

            def do_batch(srcs, b, j0, n):
                """hsum + e for blocks of n tiles starting at j0. Emits all
                products (DVE) then all PE folds; RETURNS a closure that
                emits the DVE reduces (PSUM -> e_all/hs_all directly). The
                caller runs it after the NEXT batch's products, so a
                product never queues behind a reduce that waits on PE
                folds — the FIFO chain that paced v10 at ~13us/batch."""
                prods = []
                for src3 in srcs:
                    prod = ppool.tile([P, n, D], F16, tag=f"prod{n}")
                    nc.vector.tensor_tensor(
                        out=prod, in0=src3,
                        in1=_rep_ap(w_bc16[:, b, :], n), op=ALU.mult,
                    )
                    prods.append(prod)
                hs_list = [_fold(s, n, "hs_ps") for s in srcs]
                e_list = [_fold(p, n, "e_ps") for p in prods]

                def reduces():
                    for v, ps in enumerate(hs_list):
                        cols = slice(
                            b * NJ + j0 + v * n, b * NJ + j0 + (v + 1) * n
                        )
                        nc.vector.tensor_reduce(
                            out=hs_all[:, cols], in_=ps[:, :n, :],
                            axis=mybir.AxisListType.X, op=ALU.add,
                        )
                    for v, ps in enumerate(e_list):
                        cols = slice(
                            b * NJ + j0 + v * n, b * NJ + j0 + (v + 1) * n
                        )
                        nc.vector.tensor_reduce(
                            out=e_all[:, cols], in_=ps[:, :n, :],
                            axis=mybir.AxisListType.X, op=ALU.add,
                        )

                return reduces

            # broadcast each w[b] down the partitions (K=1 fp16 matmuls, all
            # early on the PE) and evacuate on ScalarE, whose queue holds
            # only the much-later exps — so neither the PE fold stream nor
            # the DVE product stream ever waits behind an evacuation.
            for b in range(B_LOC):
                bc_ps = psum1.tile([P, D], FP, tag="bc_ps")
                nc.tensor.matmul(bc_ps, lhsT=ones16_1x128, rhs=w_rows16[:, b, :])
                nc.scalar.copy(out=w_bc16[:, b, :], in_=bc_ps)

            def finish_batch(b):
                """row max + exp(e - colmax_p); needs batch b's e columns"""
                c0 = b * NJ
                nc.vector.tensor_reduce(
                    out=ncm_all[:, b : b + 1], in_=e_all[:, c0 : c0 + NJ],
                    axis=mybir.AxisListType.X, op=ALU.max, negate=True,
                )
                nc.scalar.activation(
                    out=exp_all[:, c0 : c0 + NJ], in_=e_all[:, c0 : c0 + NJ],
                    func=AF.Exp, bias=ncm_all[:, b : b + 1], scale=1.0,
                    accum_out=pscol_all[:, b : b + 1],
                )

            pending = []  # [(reduce_closure, batch_or_None_to_finish)]
            for b in range(B_LOC):
                last = b == B_LOC - 1
                if last:
                    # quarter tiles: precise deps so the tail work starts
                    # as each quarter lands, not after the full batch
                    h3 = h[b].rearrange("(p j) d -> p j d", p=P)
                    hqs = []
                    for v in range(4):
                        hq = qpool.tile([P, QN, D], F16, tag=f"hq{v}")
                        nc.gpsimd.dma_start(
                            out=hq, in_=h3[:, v * QN : (v + 1) * QN, :]
                        )
                        hqs.append(hq)
                    r1 = do_batch(hqs[:2], b, 0, QN)
                    for red, fb in pending:
                        red()
                        if fb is not None:
                            finish_batch(fb)
                    pending = []
                    r2 = do_batch(hqs[2:], b, 2 * QN, QN)
                    r1()
                    r2()
                    finish_batch(b)
                else:
                    ht = hpool.tile([P, NJ, D], F16, tag="ht")
                    nc.gpsimd.dma_start(
                        out=ht, in_=h[b].rearrange("(p j) d -> p j d", p=P)
                    )
                    red = do_batch(
                        [ht[:, 0 : NJ // 2, :], ht[:, NJ // 2 :, :]],
                        b, 0, NJ // 2,
                    )
                    for r, fb in pending:
                        r()
                        if fb is not None:
                            finish_batch(fb)
                    pending = [(red, b)]

        # ---------------- stage 2: batched softmax combine ----------------
        with tc.tile_pool(name="psum2", bufs=1, space="PSUM") as psum2:
            # bmax[b] = max_p colmax[p, b]; ncm = -colmax
            cmT_ps = psum2.tile([B_LOC, P], FP, tag="cmT_ps")
            nc.tensor.transpose(cmT_ps, in_=ncm_all, identity=ident)
            nc.vector.tensor_reduce(
                out=bmax_sb, in_=cmT_ps, axis=mybir.AxisListType.X,
                op=ALU.min, negate=True,
            )
            bt_ps = psum2.tile([1, B_LOC], FP, tag="bt_ps")
            nc.tensor.transpose(
                bt_ps, in_=bmax_sb, identity=ident[:B_LOC, :B_LOC]
            )
            nc.vector.tensor_copy(out=bt_sb, in_=bt_ps)
            # -bmax broadcast down partitions
            nb_ps = psum2.tile([P, B_LOC], FP, tag="nb_ps")
            nc.tensor.matmul(nb_ps, lhsT=neg_1x128, rhs=bt_sb)
            nc.vector.tensor_copy(out=nb_sb, in_=nb_ps)
            # cmarg = colmax - bmax = nb - ncm
            nc.vector.tensor_tensor(
                out=cmarg, in0=nb_sb, in1=ncm_all, op=ALU.subtract
            )
            nc.scalar.activation(out=cmexp, in_=cmarg, func=AF.Exp)
            nc.vector.tensor_tensor(
                out=pscw, in0=pscol_all, in1=cmexp, op=ALU.mult
            )
            # Z[b] = sum_p pscw[p, b]
            z_ps = psum2.tile([B_LOC, 1], FP, tag="z_ps")
            nc.tensor.matmul(z_ps, lhsT=pscw, rhs=ones_128x1)
            nc.vector.reciprocal(out=rcp_sb, in_=z_ps)
            rt_ps = psum2.tile([1, B_LOC], FP, tag="rt_ps")
            nc.tensor.transpose(
                rt_ps, in_=rcp_sb, identity=ident[:B_LOC, :B_LOC]
            )
            nc.vector.tensor_copy(out=rt_sb, in_=rt_ps)
            rb_ps = psum2.tile([P, B_LOC], FP, tag="rb_ps")
            nc.tensor.matmul(rb_ps, lhsT=ones_1x128, rhs=rt_sb)
            nc.vector.tensor_copy(out=rb_sb, in_=rb_ps)
            nc.vector.tensor_tensor(out=wf_sb, in0=cmexp, in1=rb_sb, op=ALU.mult)

            # c = exp_all * hs_all * wf[p, b]  (wf broadcast over j)
            nc.vector.tensor_tensor(
                out=cbuf, in0=exp_all, in1=hs_all, op=ALU.mult
            )
            wf_rep = bass.AP(
                tensor=wf_sb.tensor, offset=wf_sb.offset,
                ap=[wf_sb.ap[0], wf_sb.ap[1], [0, NJ]],
            )
            cbuf3 = bass.AP(
                tensor=cbuf.tensor, offset=cbuf.offset,
                ap=[cbuf.ap[0], [NJ, B_LOC], [1, NJ]],
            )
            nc.vector.tensor_tensor(
                out=cbuf3, in0=cbuf3, in1=wf_rep, op=ALU.mult
            )
            # c[b, p*16 + j] = cbuf[p, b*16 + j]. The DRAM pattern is 1024
            # 64B segments (RMW-penalized), so split across both HWDGE
            # rings plus the (now idle) SWDGE ring to cut the serial time.
            for lane, (engine, b0r, b1r) in enumerate(
                [(nc.sync, 0, 3), (nc.scalar, 3, 6), (nc.gpsimd, 6, 8)]
            ):
                engine.dma_start(
                    out=c_out[b0r:b1r].rearrange("b (p j) -> p b j", p=P),
                    in_=cbuf[:, b0r * NJ : b1r * NJ],
                )


_CACHE = {}


def _build():
    if "nc" not in _CACHE:
        nc = bacc.Bacc(
            "TRN2", target_bir_lowering=False, debug=False, num_devices=N_CORES
        )
        with tile.TileContext(nc) as tc:
            _emit(nc, tc)
        nc.compile()
        _CACHE["nc"] = nc
    return _CACHE["nc"]


def kernel(s, h, phi_w, phi_b, psi_w, psi_b=None, **_unused):
    s = np.ascontiguousarray(np.asarray(s, dtype=np.float32))
    h = np.ascontiguousarray(np.asarray(h, dtype=np.float32))
    phi_w = np.ascontiguousarray(np.asarray(phi_w, dtype=np.float32))
    phi_b = np.ascontiguousarray(np.asarray(phi_b, dtype=np.float32))
    psi_w = np.ascontiguousarray(np.asarray(psi_w, dtype=np.float32))

    nc = _build()
    in_maps = [
        {
            "s": s[i * B_LOC : (i + 1) * B_LOC],
            "h": h[i * B_LOC : (i + 1) * B_LOC],
            "phi_w": phi_w,
            "phi_b": phi_b,
            "psi_w": psi_w,
        }
        for i in range(N_CORES)
    ]
    res = bass_utils.run_bass_kernel_spmd(nc, in_maps, core_ids=list(range(N_CORES)))
    return np.concatenate(
        [res.results[i]["c"] for i in range(N_CORES)], axis=0
    ).astype(np.float32)
